# revision 29
# baseline (speedup 1.0000x reference)
"""Multi-head attention (B=4, S=2048, D=1024, H=16) on 8 TRN2 NeuronCores.

Sharding: core c = 2*b + g handles batch b (of 4) and head-group g (of 2,
8 heads / 512 model dims each).  Per core:
  - Q/K projections in fp8e4m3 with perf_mode=DoubleRow (2 contraction
    blocks per matmul, 0.5 cyc/row); weights pre-scaled x16 on host, the
    x256 folded out via the exp() input scale.  V projection in bf16.
  - attention for its 8 heads processed in PAIRS (A on SBUF partitions
    0-63, B on 64-127) in transposed-scores layout (scoresT[k, q]):
    the two scores matmuls use disjoint PE row-groups (tile_position
    (0,0) / (64,0)) so they stream concurrently through the array;
    exp_A / exp_B alternate on the scalar engine (the critical resource:
    ~250us of exp) and overlap the PV matmuls.  Softmax denominator via
    a ones-column appended to V; no max subtraction (scores ~N(0,0.08^2)
    after scaling, exp cannot overflow).
  - PSUM: 2 score slots (4 banks) + pv pair / projection accumulators
    (4 banks) exactly fill the 8 banks.
  - output projection partial over its 512 model dims, interleaved with
    attention; partials ReduceScatter'd pairwise in 8 chunks so the
    collective overlaps compute.
Host: pre-transposes inputs/weights, feeds per-core shards, reassembles
the full [4, 2048, 1024] fp32 output (chunked-RS row interleaving: core
2b+g holds rows 256*ch + [128*g, 128*(g+1)) of batch b for ch in 0..7).
"""

import numpy as np
import ml_dtypes

import concourse.bass as bass
import concourse.mybir as mybir
import concourse.tile as tile
from concourse import bacc
from concourse.bass_utils import run_bass_kernel_spmd

N_CORES = 8
S = 2048          # sequence length
D = 1024          # d_model
DL = 512          # local model dims (8 heads x 64)
NH = 8            # local heads
DH = 64           # head dim
WSCALE = 16.0     # host pre-scale on wq/wk (fp8 range use)
SCALE = 1.0 / 32.0 / (WSCALE * WSCALE)  # 1/sqrt(d_model) / x256

F32 = mybir.dt.float32
BF16 = mybir.dt.bfloat16
F8 = mybir.dt.float8e4

_NC_CACHE = None


def _build_nc(repeat=1, phases="abc", collective=True, overlap_c=True):
    nc = bacc.Bacc("TRN2", target_bir_lowering=False, debug=False,
                   num_devices=N_CORES)

    xq = nc.dram_tensor("xq", [D, S], F8, kind="ExternalInput")
    xk = nc.dram_tensor("xk", [D, S], F8, kind="ExternalInput")
    xv = nc.dram_tensor("xv", [D, S], BF16, kind="ExternalInput")
    wqt = nc.dram_tensor("wqt", [D, DL], F8, kind="ExternalInput")
    wkt = nc.dram_tensor("wkt", [D, DL], F8, kind="ExternalInput")
    wvt = nc.dram_tensor("wvt", [D, DL], BF16, kind="ExternalInput")
    wot = nc.dram_tensor("wot", [DL, D], BF16, kind="ExternalInput")
    y = nc.dram_tensor("y", [S // 2, D], F32, kind="ExternalOutput")

    ypart = nc.dram_tensor("ypart", [S, D], F32)
    yrs = nc.dram_tensor("yrs", [S // 2, D], F32)

    with tile.TileContext(nc) as tc:
        with (
            tc.tile_pool(name="big", bufs=20) as big,       # xv / khT / qhT / attn
            tc.tile_pool(name="xf8", bufs=6) as xf8p,       # fp8 xq/xk pair tiles
            tc.tile_pool(name="wp", bufs=1) as wpool,       # wv (bf16)
            tc.tile_pool(name="wf8", bufs=2) as wf8p,       # wq/wk fp8
            tc.tile_pool(name="wop", bufs=1) as wopool,     # woT
            tc.tile_pool(name="vhp", bufs=16) as vhp,       # vh | ones
            tc.tile_pool(name="expp", bufs=12) as expp,      # exp(scores)
            tc.tile_pool(name="pvsp", bufs=2) as pvsp,      # pv psum drain
            tc.tile_pool(name="rcp", bufs=2) as rcp,        # reciprocal row
            tc.tile_pool(name="rbp", bufs=2) as rbp,        # bcast reciprocal
            tc.tile_pool(name="stgp", bufs=2) as stgp,      # psum->dram staging
            tc.tile_pool(name="scp", bufs=2, space="PSUM") as scp,  # scores
            tc.tile_pool(name="pvp", bufs=2, space="PSUM") as pvp,  # pv + accs
        ):
            for rep in range(repeat):
                pfx = f"r{rep}_"
                # ------------- K/Q projections (fp8 DoubleRow) -------------
                # out[dl_block, seq] = sum_kc2 wT[kc2,:,dl].T @ xT[kc2,:,seq]
                # khT/qhT tile mc holds heads 2mc (partitions 0-63) and
                # 2mc+1 (64-127), bf16, scaled x256 (folded into exp scale).
                khT_sb, qhT_sb = [None] * 4, [None] * 4
                for name, wdram, xdram, dest in (
                    ("k", wkt, xk, khT_sb),
                    ("q", wqt, xq, qhT_sb),
                ):
                    dma_eng = nc.sync if name == "k" else nc.scalar
                    wq_sb = wf8p.tile([128, 4, 2, DL], F8, tag="wf8",
                                      name=f"{pfx}w_{name}")
                    dma_eng.dma_start(
                        out=wq_sb[:],
                        in_=wdram[:].rearrange("(kc two p) m -> p kc two m",
                                               p=128, two=2),
                    )
                    x_sb = []
                    for kc2 in range(4):
                        xt = xf8p.tile([128, 2, S], F8, tag="xf8",
                                       name=f"{pfx}x{name}_{kc2}")
                        dma_eng.dma_start(
                            out=xt[:],
                            in_=xdram[kc2 * 256:(kc2 + 1) * 256, :].rearrange(
                                "(two p) s -> p two s", p=128),
                        )
                        x_sb.append(xt)
                    for mc in range(4):
                        pt = big.tile([128, S], BF16, tag="big",
                                      name=f"{pfx}{name}hT_{mc}")
                        dest[mc] = pt
                        for nt2 in range(2):
                            acc = scp.tile([128, 1024], F32, tag="sc",
                                           name=f"{pfx}ps{name}_{mc}_{nt2}")
                            for half in range(2):
                                nt = 2 * nt2 + half
                                for kc2 in range(4):
                                    nc.tensor.matmul(
                                        acc[:, half * 512:(half + 1) * 512],
                                        wq_sb[:, kc2, :, mc * 128:(mc + 1) * 128],
                                        x_sb[kc2][:, :, nt * 512:(nt + 1) * 512],
                                        start=(kc2 == 0),
                                        stop=(kc2 == 3),
                                        perf_mode=mybir.MatmulPerfMode.DoubleRow,
                                    )
                            ceng = nc.vector if (mc + nt2) % 2 == 0 else nc.scalar
                            if ceng is nc.vector:
                                ceng.tensor_copy(
                                    pt[:, nt2 * 1024:(nt2 + 1) * 1024], acc[:])
                            else:
                                ceng.copy(
                                    pt[:, nt2 * 1024:(nt2 + 1) * 1024], acc[:])

                # ---------------- V projection (bf16, x-stationary) --------
                w_sb = wpool.tile([128, 8, DL], BF16, tag="w", name=f"{pfx}w_v")
                nc.sync.dma_start(
                    out=w_sb[:], in_=wvt[:].rearrange("(kc p) m -> p kc m", p=128)
                )
                x_sb = []
                for kc in range(8):
                    xt = big.tile([128, S], BF16, tag="big", name=f"{pfx}xv_{kc}")
                    nc.sync.dma_start(out=xt[:], in_=xv[kc * 128:(kc + 1) * 128, :])
                    x_sb.append(xt)
                vh_sb = []
                for st in range(16):
                    acc = pvp.tile([128, 512], F32, tag="pv", name=f"{pfx}psv_{st}")
                    for kc in range(8):
                        nc.tensor.matmul(
                            acc[:],
                            x_sb[kc][:, st * 128:(st + 1) * 128],
                            w_sb[:, kc, :],
                            start=(kc == 0),
                            stop=(kc == 7),
                        )
                    vt = vhp.tile([128, NH, DH + 1], BF16, tag="vh", name=f"{pfx}vh_{st}")
                    nc.vector.tensor_copy(
                        vt[:, :, 0:DH], acc[:].rearrange("p (h d) -> p h d", d=DH)
                    )
                    nc.vector.memset(vt[:, :, DH:DH + 1], 1.0)
                    vh_sb.append(vt)

                # woT load (bf16): [512, 1024] -> [128, 4, 1024];
                # needed only by the output projection, so loaded last
                wo_sb = wopool.tile([128, 4, D], BF16, tag="wo", name=f"{pfx}wo_sb")
                nc.scalar.dma_start(
                    out=wo_sb[:], in_=wot[:].rearrange("(t p) n -> p t n", p=128)
                )

                # ---------------- attention ----------------
                if "b" not in phases:
                    continue
                attn_sb = [
                    big.tile([128, S], BF16, tag="big", name=f"{pfx}attn_{t}")
                    for t in range(4)
                ]

                def emit_c(qb_lo, qb_hi):
                    # output projection + chunked ReduceScatter for q rows
                    # [128*qb_lo, 128*qb_hi)
                    if "c" not in phases:
                        return
                    for qb in range(qb_lo, qb_hi):
                        acc = pvp.tile([128, 1024], F32, tag="pv",
                                       name=f"{pfx}psy_{qb}")
                        for nt in range(2):
                            for t in range(4):
                                nc.tensor.matmul(
                                    acc[:, nt * 512:(nt + 1) * 512],
                                    attn_sb[t][:, qb * 128:(qb + 1) * 128],
                                    wo_sb[:, t, nt * 512:(nt + 1) * 512],
                                    start=(t == 0),
                                    stop=(t == 3),
                                )
                        for nt in range(2):
                            st = stgp.tile([128, 512], F32, tag="ystg",
                                           name=f"{pfx}sty_{qb}_{nt}")
                            nc.vector.tensor_copy(
                                st[:], acc[:, nt * 512:(nt + 1) * 512])
                            nc.sync.dma_start(
                                out=ypart[qb * 128:(qb + 1) * 128,
                                          nt * 512:(nt + 1) * 512],
                                in_=st[:],
                            )
                        if qb % 4 == 3:
                            ch = qb // 4
                            if collective:
                                nc.gpsimd.collective_compute(
                                    "ReduceScatter",
                                    mybir.AluOpType.add,
                                    replica_groups=[[0, 1], [2, 3], [4, 5], [6, 7]],
                                    ins=[ypart[512 * ch:512 * (ch + 1), :].opt()],
                                    outs=[yrs[256 * ch:256 * (ch + 1), :].opt()],
                                )
                            elif ch < 2:
                                nc.sync.dma_start(
                                    out=y[512 * ch:512 * (ch + 1), :],
                                    in_=ypart[512 * ch:512 * (ch + 1), :],
                                )

                # persistent score tiles: allocated once, reused for all
                # 128 kb-steps (WAR deps replace per-step slot churn);
                # released before the qt=1 output projection needs nothing
                # from scp (outproj accs live in pvp)
                sc_pair = [
                    scp.tile([128, 1024], F32, tag="sc", name=f"{pfx}sc_{p}")
                    for p in range(2)
                ]
                for qt in range(2):
                    for mc in range(4):
                        kh = khT_sb[mc]
                        qsl = qhT_sb[mc]
                        # head pair: A = 2mc (partitions 0-63),
                        #            B = 2mc+1 (64-127); the two scores
                        # matmuls land on disjoint PE row groups.
                        pv = [
                            pvp.tile([65, 1024], F32, tag="pv",
                                     name=f"{pfx}pv_{mc}_{qt}_{p}")
                            for p in range(2)
                        ]
                        for kb in range(16):
                            exs = []
                            for p in range(2):
                                sc = sc_pair[p]
                                for half in range(2):
                                    nc.tensor.matmul(
                                        sc[:, half * 512:(half + 1) * 512],
                                        kh[64 * p:64 * p + 64,
                                           kb * 128:(kb + 1) * 128],
                                        qsl[64 * p:64 * p + 64,
                                            qt * 1024 + half * 512:
                                            qt * 1024 + (half + 1) * 512],
                                        start=True,
                                        stop=True,
                                    )
                                ex = expp.tile([128, 1024], BF16, tag="exp",
                                               name=f"{pfx}ex_{mc}_{qt}_{kb}_{p}")
                                nc.scalar.activation(
                                    ex[:], sc[:],
                                    mybir.ActivationFunctionType.Exp,
                                    scale=SCALE,
                                )
                                exs.append(ex)
                            for p in range(2):
                                for half in range(2):
                                    nc.tensor.matmul(
                                        pv[p][:, half * 512:(half + 1) * 512],
                                        vh_sb[kb][:, 2 * mc + p, :],
                                        exs[p][:, half * 512:(half + 1) * 512],
                                        start=(kb == 0),
                                        stop=(kb == 15),
                                    )
                        # drain + normalize the pair
                        for p in range(2):
                            pvs = pvsp.tile([65, 1024], F32, tag="pvs",
                                            name=f"{pfx}pvs_{mc}_{qt}_{p}")
                            nc.vector.tensor_copy(pvs[:], pv[p][:])
                            rc = rcp.tile([1, 1024], F32, tag="rc",
                                          name=f"{pfx}rc_{mc}_{qt}_{p}")
                            nc.vector.reciprocal(rc[:], pvs[64:65, :])
                            rb = rbp.tile([64, 1024], F32, tag="rb",
                                          name=f"{pfx}rb_{mc}_{qt}_{p}")
                            nc.gpsimd.partition_broadcast(rb[:], rc[:])
                            nc.vector.tensor_mul(
                                attn_sb[mc][64 * p:64 * p + 64,
                                            qt * 1024:(qt + 1) * 1024],
                                pvs[0:64, :], rb[:]
                            )
                        # qt=0's output projection dribbles out 2 q-blocks
                        # after each qt=1 pair so it never hogs the pv slots
                        if overlap_c and qt == 1:
                            emit_c(2 * mc, 2 * mc + 2)
                if overlap_c:
                    emit_c(8, 16)
                else:
                    emit_c(0, 16)
                if collective:
                    # yrs -> y via SBUF staging (DRAM->DRAM DMA is slow);
                    # emitted last so the RS-completion waits never block
                    # earlier staging DMAs on the in-order queues
                    for sub in range(8):
                        yst = pvsp.tile([128, 1024], F32, tag="pvs",
                                        name=f"{pfx}yst_{sub}")
                        nc.sync.dma_start(
                            out=yst[:],
                            in_=yrs[128 * sub:128 * (sub + 1), :],
                        )
                        nc.sync.dma_start(
                            out=y[128 * sub:128 * (sub + 1), :],
                            in_=yst[:],
                        )

    nc.finalize()
    return nc


def make_in_maps(q, k, v, wq, wv, wk, wo):
    """Per-core input shards. Core c=2b+g: batch b, head-group g."""
    f8 = mybir.dt.np(F8)
    bf = ml_dtypes.bfloat16
    in_maps = []
    for c in range(N_CORES):
        b, g = c // 2, c % 2
        sl = slice(DL * g, DL * (g + 1))
        in_maps.append({
            "xq": np.ascontiguousarray(q[b].T).astype(f8),
            "xk": np.ascontiguousarray(k[b].T).astype(f8),
            "xv": np.ascontiguousarray(v[b].T).astype(bf),
            "wqt": (np.ascontiguousarray(wq[sl, :].T) * WSCALE).astype(f8),
            "wkt": (np.ascontiguousarray(wk[sl, :].T) * WSCALE).astype(f8),
            "wvt": np.ascontiguousarray(wv[sl, :].T).astype(bf),
            "wot": np.ascontiguousarray(wo[:, sl].T).astype(bf),
        })
    return in_maps


def kernel(q, k, v, wq, wk, wv, wo, _res_hook=None):
    q = np.asarray(q, dtype=np.float32)
    k = np.asarray(k, dtype=np.float32)
    v = np.asarray(v, dtype=np.float32)
    wq = np.asarray(wq, dtype=np.float32)
    wk = np.asarray(wk, dtype=np.float32)
    wv = np.asarray(wv, dtype=np.float32)
    wo = np.asarray(wo, dtype=np.float32)
    B = q.shape[0]

    nc = _get_nc()
    in_maps = make_in_maps(q, k, v, wq, wv, wk, wo)

    res = run_bass_kernel_spmd(nc, in_maps, list(range(N_CORES)))
    if _res_hook is not None:
        _res_hook(res)

    out = np.empty((B, S, D), dtype=np.float32)
    for c in range(N_CORES):
        b, g = c // 2, c % 2
        yc = res.results[c]["y"]
        for ch in range(4):
            out[b, 512 * ch + 256 * g:512 * ch + 256 * (g + 1), :] = \
                yc[256 * ch:256 * (ch + 1), :]
    return out


def _get_nc():
    global _NC_CACHE
    if _NC_CACHE is None:
        _NC_CACHE = _build_nc()
    return _NC_CACHE
